# revision 17
# baseline (speedup 1.0000x reference)
"""Trainium2 Bass kernel for nn_AttentionModel (sparse_attention).

Reference computation:
    x = emb_table[tokens]                  # [B,S,D]
    scores = x @ x^T per batch             # [B,S,S]
    out = softmax(scores) @ x              # [B,S,D]
    logits = out[:, 0, :] @ cls_w.T + cls_b

Only row 0 of the attention output is used, so per batch element we only
need attention row 0:
    q = x[0]; s = X q; a = exp(s); logits = (a^T X / sum(a)) cls_w^T + cls_b
This turns ~275 GFLOP of full attention into an embedding gather plus
~4 MFLOP per batch element.

Sharding: data-parallel over batch. 8 cores x 4 batch elements; emb_table
and classifier weights replicated; no collectives.
"""

import numpy as np

import concourse.bass as bass
import concourse.mybir as mybir
import concourse.tile as tile
import tile_patch
from concourse.bass_utils import run_bass_kernel_spmd

B, S, D, V, C = 32, 2048, 512, 32000, 2
N_CORES = 8
BPC = B // N_CORES          # batch elements per core
SBLK = S // 128             # 16 free-dim blocks of gathered tokens

F32 = mybir.dt.float32
I32 = mybir.dt.int32

_CACHE: dict = {}


def _build_nc(split: bool = True) -> bass.Bass:
    nc = bass.Bass()
    emb_d = nc.dram_tensor("emb", [V, D], F32, kind="ExternalInput")
    tok_d = nc.dram_tensor("tok", [BPC, 128, SBLK], I32, kind="ExternalInput")
    cw_d = nc.dram_tensor("cls_w", [1, C * D], F32, kind="ExternalInput")
    cb_d = nc.dram_tensor("cls_b", [1, C], F32, kind="ExternalInput")
    out_d = nc.dram_tensor("out", [BPC, C], F32, kind="ExternalOutput")

    mult = mybir.AluOpType.mult
    add = mybir.AluOpType.add
    EXP = mybir.ActivationFunctionType.Exp

    with tile.TileContext(nc) as tc:
        with (
            tc.tile_pool(name="const", bufs=1) as constp,
            tc.tile_pool(name="xp", bufs=2) as xp,
            tc.tile_pool(name="sp", bufs=2) as sp,
            tc.tile_pool(name="jp", bufs=2) as jp,
            tc.tile_pool(name="tp", bufs=2) as tp,
            tc.tile_pool(name="ps", bufs=2, space="PSUM") as pp,
        ):
            ones1 = constp.tile([1, 128], F32)
            nc.vector.memset(ones1[:], 1.0)
            ones128 = constp.tile([128, 1], F32)
            nc.vector.memset(ones128[:], 1.0)
            cw = constp.tile([1, C, D], F32)
            nc.sync.dma_start(cw[:], cw_d[:, :])
            cb = constp.tile([1, C], F32)
            nc.sync.dma_start(cb[:], cb_d[:, :])
            idx = constp.tile([128, BPC, SBLK], I32)
            for b in range(BPC):
                nc.sync.dma_start(idx[:, b, :], tok_d[b, :, :])

            for b in range(BPC):
                # Gather the 2048 embedding rows for this batch element.
                # Token t lands on partition t%128, free block t//128; one
                # indirect DMA per 128-token block (one index per partition).
                x = xp.tile([128, SBLK, D], F32, tag="x")
                for j in range(SBLK):
                    nc.gpsimd.indirect_dma_start(
                        out=x[:, j, :],
                        out_offset=None,
                        in_=emb_d[:, :],
                        in_offset=bass.IndirectOffsetOnAxis(
                            ap=idx[:, b, j : j + 1], axis=0
                        ),
                    )

                # Broadcast q = x[token 0] to all 128 partitions via a K=1
                # outer-product matmul: ones[1,128]^T @ x[0:1, 0, :].
                qb = pp.tile([128, D], F32, tag="qb")
                nc.tensor.matmul(qb[:], ones1[:], x[0:1, 0, :], start=True, stop=True)

                # Scores s[t] = <x_t, q>: fused multiply+reduce per block
                # ((x*1) * q with accum_out = row sums).
                s = sp.tile([128, SBLK], F32, tag="s")
                for j in range(SBLK):
                    junk = jp.tile([128, D], F32, tag="junk")
                    nc.vector.scalar_tensor_tensor(
                        out=junk[:],
                        in0=x[:, j, :],
                        scalar=1.0,
                        in1=qb[:],
                        op0=mult,
                        op1=mult,
                        accum_out=s[:, j : j + 1],
                    )

                # a = exp(s) (scores are O(0.2): no max subtraction needed),
                # with fused per-partition row sums for the softmax denom.
                e = sp.tile([128, SBLK], F32, tag="e")
                zcol = sp.tile([128, 1], F32, tag="zcol")
                nc.scalar.activation(e[:], s[:], EXP, accum_out=zcol[:])

                # pooled = a^T X (unnormalized), accumulated over blocks.
                pooled = pp.tile([1, D], F32, tag="pooled")
                for j in range(SBLK):
                    nc.tensor.matmul(
                        pooled[:],
                        e[:, j : j + 1],
                        x[:, j, :],
                        start=(j == 0),
                        stop=(j == SBLK - 1),
                    )

                # Z = sum over partitions of zcol.
                zps = pp.tile([1, 1], F32, tag="z")
                nc.tensor.matmul(zps[:], zcol[:], ones128[:, :], start=True, stop=True)

                psb = tp.tile([1, D], F32, tag="psb")
                nc.scalar.copy(psb[:], pooled[:])
                zsb = tp.tile([1, 1], F32, tag="zsb")
                nc.vector.tensor_copy(zsb[:], zps[:])
                rz = tp.tile([1, 1], F32, tag="rz")
                nc.vector.reciprocal(rz[:], zsb[:])

                # logits_c = <pooled, cls_w_c>
                lg = tp.tile([1, C], F32, tag="lg")
                for c in range(C):
                    junk2 = tp.tile([1, D], F32, tag="junk2")
                    nc.vector.scalar_tensor_tensor(
                        out=junk2[:],
                        in0=psb[:],
                        scalar=1.0,
                        in1=cw[:, c, :],
                        op0=mult,
                        op1=mult,
                        accum_out=lg[:, c : c + 1],
                    )

                # out = lg / Z + cls_b
                ob = tp.tile([1, C], F32, tag="ob")
                nc.vector.scalar_tensor_tensor(
                    ob[:], lg[:], rz[:], cb[:], op0=mult, op1=add
                )
                nc.sync.dma_start(out_d[b : b + 1, :], ob[:])

    nc.finalize()
    if split:
        tile_patch.split_multiwaits(nc)
    return nc


def _wrap_tokens(tokens_row: np.ndarray) -> np.ndarray:
    """[S] int tokens -> [128, SBLK] int32; token t at [t%128, t//128]."""
    return np.ascontiguousarray(tokens_row.reshape(SBLK, 128).T.astype(np.int32))


def get_nc() -> bass.Bass:
    if "nc" not in _CACHE:
        _CACHE["nc"] = _build_nc()
    return _CACHE["nc"]


def make_in_maps(tokens, emb_table, cls_w, cls_b):
    tokens = np.asarray(tokens)
    emb = np.ascontiguousarray(np.asarray(emb_table, dtype=np.float32))
    cw = np.ascontiguousarray(np.asarray(cls_w, dtype=np.float32)).reshape(1, C * D)
    cb = np.ascontiguousarray(np.asarray(cls_b, dtype=np.float32)).reshape(1, C)
    idx_all = np.stack([_wrap_tokens(tokens[b]) for b in range(B)])  # [B,128,IDXW]
    in_maps = []
    for core in range(N_CORES):
        in_maps.append(
            {
                "emb": emb,
                "tok": idx_all[core * BPC : (core + 1) * BPC],
                "cls_w": cw,
                "cls_b": cb,
            }
        )
    return in_maps


def kernel(tokens, emb_table, cls_w, cls_b) -> np.ndarray:
    nc = get_nc()
    in_maps = make_in_maps(tokens, emb_table, cls_w, cls_b)
    res = run_bass_kernel_spmd(nc, in_maps, core_ids=list(range(N_CORES)))
    outs = [res.results[c]["out"] for c in range(N_CORES)]
    return np.concatenate(outs, axis=0).astype(np.float32)
